# revision 112
# baseline (speedup 1.0000x reference)
"""Trainium2 Bass kernel for nn_LinearAttention (gated linear attention).

Math (per reference):
    qkv = x @ Wqkv.T ; q,k,v = split(qkv); q,k = elu(.)+1
    per (b,h): running_kv[t]  = d*running_kv[t-1]  + k[t]*v[t]   (elementwise, D=64)
               running_ksum[t]= d*running_ksum[t-1]+ k[t]
    den = clip(sum_d(q*running_ksum), 1e-6); out = q*running_kv/den
    g = sigmoid(out @ Wgate.T + bgate); out = g*out + (1-g)*v
    y = out @ Wout.T

Implementation strategy (8 NeuronCores, SPMD, no collectives):
  - Token-parallel: core c handles batch b=c//2, T-half h=c%2 (2048 tokens)
    plus a 128-token halo to warm the decay scan (0.95^128 ~ 1.4e-3, well
    under the error budget).  Half 0 gets a zero halo + k-mask so its scan
    state is exactly 0 at t=0.
  - Everything on-chip is [feature(partition), token(free)]; the host
    pre-transposes x and the weights so no on-chip transpose is needed.
  - The host scales Wqkv by 32 (exact in bf16); every activation rides
    that x32 scale.  phi drains straight from PSUM via
    32*phi(x) = max(ps+32, 32*min(exp(ps/32),1)) (ps = 32x), fused into
    one custom DVE op (LA_PHI_TAIL).  The pipeline is linear/ratio in
    the scale so it cancels everywhere except the gate sigmoid (ACT
    scale) and final y copy (ACT scale).
  - The q,k projection sections and the gate matmul run in fp8
    DoubleRow (2x PE rate, halved instruction count).  Numpy simulation
    of e4m3 quantization showed q/k are error-free to ~4e-3 (phi + the
    den ratio + the decay scan all damp it) while v/out quantization
    blows the 2e-2 budget, so the v section and out matmul stay bf16.
  - Decay scans run as custom DVE ops in cumsum form (see
    _register_dve_ops): LA_XP_KV fuses the k*v product with the
    (1/d)^(t+1) weighting, LA_CSUM adds the cross-iteration carry.  Each
    op runs ~1 elem/cycle/partition vs the stock tensor_tensor_scan's
    ~2.7ns/elem.  The d^(t+1) rescale of the running sums CANCELS
    between the attention numerator (q*cs_kv) and denominator (q.cs_ks),
    so qc/prods are plain unscaled tensor_tensor multiplies -- ridden on
    gpsimd where the consumer has a full iteration of slack (prods ->
    den -> bc, and qc -> oa except in the last iteration).  Iter-1 scans
    are 640 wide (128 halo + 512), later iters 512, chained via [128,1]
    state tiles (gpsimd).
  - den: 0/1 block-diag selector matmul -> [16,512] psum; clip, fast
    approx reciprocal, cast bf16; broadcast back to 128 partitions via a
    bf16 selector matmul (bc).
  - Two-level software pipeline: iter i ends with kv scans + qc=q1*ckv;
    the dependent [bc matmul, oa=qc*bc, oa8 fp8 cast, dls=oa-v] block,
    the gate/mix, and the out matmul for iter i all run inside iter i+1,
    overlapped with its qkv sections.  This keeps the PE queue free of
    head-of-line blocking at iteration boundaries.
"""

import sys

for _p in ('/opt/trn_rl_repo', '/root/.axon_site'):
    if _p not in sys.path:
        sys.path.insert(0, _p)

from contextlib import ExitStack

import ml_dtypes
import numpy as np

import concourse.tile as tile
from concourse import bacc, mybir
from concourse.bass_utils import run_bass_kernel_spmd

F32 = mybir.dt.float32
BF16 = mybir.dt.bfloat16
FP8 = mybir.dt.float8e4
AL = mybir.AluOpType
AF = mybir.ActivationFunctionType
DR = mybir.MatmulPerfMode.DoubleRow


def _register_dve_ops():
    """Register this kernel's fused DVE ops (idempotent).

    The decay scan out[t] = d*out[t-1] + x[t] is computed closed-form as
    cs[t] = init + cumsum(x[t] * d^-(t+1)); out[t] = d^(t+1) * cs[t].
    The unscaled cs rides between ops in bf16 (range ~d^-640 * |x| ~ 1e19,
    well inside bf16/f32 range; f32 fold error ~2^-24 * sum-window).  The
    d^(t+1) rescale is fused into the q* consumer (LA_MULSCAN).
    """
    import numpy as np
    import concourse.dve_ops as dvo
    from concourse.dve_spec import (Spec, Src0, Src1, C0, C1, C2, One, maxx,
                                    minn, Scan, lower, AluOp, _has_src1)
    from concourse.dve_uop import DveOpSpec

    def reg(name, body, ref, accum=None):
        if name in dvo._SUB_OPCODE_FOR_NAME:
            return next(o for o in dvo.OPS if o.name == name)
        spec = Spec(body=body, reference=ref, accum=accum)
        row = dvo._CUSTOM_DVE_ROW_BASE + len(dvo.OPS)
        shas = {}
        for ver in ("v3", "v4"):
            try:
                s = DveOpSpec(name=name, opcode=row, uops=lower(spec, ver=ver),
                              rd1_en=_has_src1(spec))
                shas[ver] = s.sha(ver)
            except Exception:
                pass
        op = dvo.DveOp(name, spec, subdim=False, uops_sha=shas)
        dvo.OPS.append(op)
        dvo._SUB_OPCODE_FOR_NAME[name] = row
        dvo.CUSTOM_DVE_SPECS[name] = spec
        return op

    def cumprod(s0, shape):
        return np.cumprod(np.broadcast_to(s0, shape), axis=-1)

    # out = max(ps + S, min(e, 1)*S) = S*phi(ps/S), e = exp(ps/S)
    phi = reg("LA_PHI_TAIL", maxx(Src0 + C0, minn(Src1, One) * C0),
              lambda in0, in1, s0: np.maximum(in0 + s0,
                                              np.minimum(in1, 1.0) * s0))
    # out = k*v * (1/d)^(t+1);  s0 = 1/d;  accum_out = sum(out)
    xp = reg("LA_XP_KV", Src0 * Src1 * Scan(AluOp.MULTIPLY, C0, init=None),
             lambda in0, in1, s0: in0 * in1 * cumprod(s0, in0.shape),
             accum=AluOp.ADD)
    # out = k * (1/d)^(t+1);  accum_out = sum(out)
    xpk = reg("LA_XP_K", Src0 * Scan(AluOp.MULTIPLY, C0, init=None),
              lambda in0, s0: in0 * cumprod(s0, in0.shape),
              accum=AluOp.ADD)
    # cs[t] = s0 + cumsum(in0)
    csum = reg("LA_CSUM", Scan(AluOp.ADD, Src0, init=C0),
               lambda in0, s0: s0 + np.cumsum(in0, axis=-1))
    # out = q * cs * (s1 * d^(t+1));  s0 = d, s1 = d^offset
    mul = reg("LA_MULSCAN",
              Src0 * Src1 * Scan(AluOp.MULTIPLY, C0, init=C1),
              lambda in0, in1, s0, s1: in0 * in1 * s1 * cumprod(s0, in0.shape))
    # out = q * (s0 + cumsum(y)) * d^(t+1);  in0 = y, in1 = q, s0 = carry,
    # s1 = d  (fused csum+mulscan, zero-offset slices only)
    csq = reg("LA_CSQ",
              Src1 * Scan(AluOp.ADD, Src0, init=C0)
              * Scan(AluOp.MULTIPLY, C1, init=None),
              lambda in0, in1, s0, s1: in1 * (s0 + np.cumsum(in0, axis=-1))
              * cumprod(s1, in0.shape))
    # out = 1/(1+in0) via the BITWISE_NOT reciprocal seed + 2 inline NR
    # passes (clone of RECIPROCAL_APPROX_FAST with a 1+x pre-add): turns
    # sigmoid(z) into Exp(-z) + this op, keeping the ACT engine on the Exp
    # table all kernel long (no Exp<->Sigmoid table reloads).
    from concourse.dve_spec import Bin
    _u = One + Src0
    _nu = Bin(AluOp.BITWISE_NOT, _u, _u)
    _g0 = _nu * C0
    _g1 = _g0 * (C1 - _u * _g0)

    def _ref_sigr(in0, s0, s1):
        u = (1.0 + in0).astype(np.float32)
        nu = (~u.view(np.int32)).view(np.float32)
        y0 = nu * s0
        return y0 * (s1 - u * y0)
    # single NR pass (~0.4% rel, same as g's bf16 rounding); 2 passes
    # would need 9 ALU stages
    sigr = reg("LA_SIGR", _g1, _ref_sigr)
    return phi, xp, xpk, csum, mul, csq, sigr

B, T, HID = 4, 4096, 1024
H, D = 16, 64
OD = 3 * HID
NK = HID // 128            # 8 contraction tiles
NH = HID // 128            # 8 tiles per q/k/v section
HALF_T = T // 2            # 2048 out tokens per core
HALO = 128
TLOC = HALO + HALF_T       # 2176
WG = 512                   # out-token group width
NG = HALF_T // WG          # 4 iterations
W1 = HALO + WG             # 640: iter-1 scan width

S = 32.0                   # activation scale riding the pipeline
OSC = 1.0 / 8.0            # oa -> fp8 cast scale (4*att, safely < 240)
GS = 1.0 / (S * S * OSC)   # gate sigmoid descale = 1/128

_cache = {}


def _build_nc():
    (PHI_OP, XP_OP, XPK_OP, CSUM_OP, MUL_OP, CSQ_OP,
     SIGR_OP) = _register_dve_ops()
    nc = bacc.Bacc("TRN2", target_bir_lowering=False, debug=False)

    xT = nc.dram_tensor("xT", [HID, TLOC], BF16, kind="ExternalInput")
    xT8 = nc.dram_tensor("xT8", [HID, TLOC], FP8, kind="ExternalInput")
    # q,k projection weights, fp8 DoubleRow-packed: row 512*sec+128*kp+p,
    # col i*HID+m = (Wqkv.T*S)[256*kp+128*i+p, HID*sec+m]
    wqk8 = nc.dram_tensor("wqk8", [HID, 2 * HID], FP8, kind="ExternalInput")
    wvT = nc.dram_tensor("wvT", [HID, HID], BF16, kind="ExternalInput")
    wg8 = nc.dram_tensor("wg8", [HID, HID], FP8, kind="ExternalInput")
    woutT = nc.dram_tensor("woutT", [HID, HID], BF16, kind="ExternalInput")
    dec_c = nc.dram_tensor("dec_c", [128, NH], F32, kind="ExternalInput")
    # decay powers: cols [0:NH)=1/d, [NH:2NH)=d^HALO, [2NH:3NH)=d^WG,
    # [3NH:4NH)=d^W1  (per head-pair partition layout, like dec_c)
    decp_c = nc.dram_tensor("decp_c", [128, 4 * NH], F32, kind="ExternalInput")
    mask_c = nc.dram_tensor("mask_c", [128, 1], F32, kind="ExternalInput")
    densel = nc.dram_tensor("densel", [128, NH * H], BF16, kind="ExternalInput")
    bcsel = nc.dram_tensor("bcsel", [H, NH * 128], BF16, kind="ExternalInput")
    bgate_c = nc.dram_tensor("bgate_c", [128, NH], F32, kind="ExternalInput")
    # bf16 output (cast to f32 host-side): halves the y DMA drain the
    # final barrier waits on; 0.4% bf16 rounding is well inside budget
    yT = nc.dram_tensor("yT", [HID, HALF_T], BF16, kind="ExternalOutput")

    with tile.TileContext(nc) as tc, ExitStack() as ctx:
        consts = ctx.enter_context(tc.tile_pool(name="consts", bufs=1))
        wq_pool = ctx.enter_context(tc.tile_pool(name="wq", bufs=1))
        wg_pool = ctx.enter_context(tc.tile_pool(name="wgp", bufs=1))
        wo_pool = ctx.enter_context(tc.tile_pool(name="wop", bufs=1))
        xt_pool = ctx.enter_context(tc.tile_pool(name="xt", bufs=12))
        x8_pool = ctx.enter_context(tc.tile_pool(name="x8", bufs=8))
        k1_pool = ctx.enter_context(tc.tile_pool(name="k1p", bufs=8))
        q1_pool = ctx.enter_context(tc.tile_pool(name="q1p", bufs=8))
        v1_pool = ctx.enter_context(tc.tile_pool(name="v1p", bufs=16))
        et_pool = ctx.enter_context(tc.tile_pool(name="et", bufs=3))
        xps_pool = ctx.enter_context(tc.tile_pool(name="xps", bufs=2))
        cum_pool = ctx.enter_context(tc.tile_pool(name="cum", bufs=1))
        st_pool = ctx.enter_context(tc.tile_pool(name="st", bufs=2))
        pr_pool = ctx.enter_context(tc.tile_pool(name="pr", bufs=8))
        qc_pool = ctx.enter_context(tc.tile_pool(name="qcp", bufs=8))
        den_pool = ctx.enter_context(tc.tile_pool(name="den", bufs=1))
        oa_pool = ctx.enter_context(tc.tile_pool(name="oa", bufs=2))
        oa8_pool = ctx.enter_context(tc.tile_pool(name="oa8", bufs=1))
        dl_pool = ctx.enter_context(tc.tile_pool(name="dl", bufs=8))
        gt_pool = ctx.enter_context(tc.tile_pool(name="gt", bufs=2))
        mx_pool = ctx.enter_context(tc.tile_pool(name="mx", bufs=8))
        y_pool = ctx.enter_context(tc.tile_pool(name="ysb", bufs=2))
        ps_pool = ctx.enter_context(tc.tile_pool(name="ps", bufs=5, space="PSUM"))
        bc_pool = ctx.enter_context(tc.tile_pool(name="bcp", bufs=2, space="PSUM"))
        psd_pool = ctx.enter_context(tc.tile_pool(name="psd", bufs=1, space="PSUM"))

        # ---- weight/const loads: weights on the gpsimd DMA queue, x on sync
        # q (sec 0) and k (sec 1) weights are fp8 DoubleRow tiles [128, 2*HID];
        # v weights stay bf16 [128, HID] x 8.
        wqk8_s = {sec: [wq_pool.tile([128, 2 * HID], FP8, tag=f"w8{sec}_{kp}",
                                     name=f"w8_{sec}_{kp}")
                        for kp in range(NK // 2)] for sec in range(2)}
        wv_s = [wq_pool.tile([128, HID], BF16, tag=f"wv{k}", name=f"wv_{k}")
                for k in range(NK)]

        def load_wqk8_sec(sec, eng):
            for kp in range(NK // 2):
                eng.dma_start(
                    wqk8_s[sec][kp][:],
                    wqk8.ap()[512 * sec + 128 * kp:512 * sec + 128 * (kp + 1), :])

        def load_wv(eng):
            for k in range(NK):
                eng.dma_start(
                    wv_s[k][:], wvT.ap()[128 * k:128 * (k + 1), :])

        dec_s = consts.tile([128, NH], F32, tag="dec")
        mask_s = consts.tile([128, 1], F32, tag="mask")
        densel_s = consts.tile([128, NH * H], BF16, tag="densel")
        bcsel_s = consts.tile([H, NH * 128], BF16, tag="bcsel")
        bgate_s = consts.tile([128, NH], F32, tag="bg")

        decp_s = consts.tile([128, 4 * NH], F32, tag="decp")

        # mask is needed first (halo phi); dec/decp follow the x loads
        nc.sync.dma_start(mask_s[:], mask_c.ap()[:, :])

        def load_dec_consts():
            nc.sync.dma_start(dec_s[:], dec_c.ap()[:, :])
            nc.sync.dma_start(decp_s[:], decp_c.ap()[:, :])

        def load_late_consts():
            # needed first at iter-1 den / iter-2 gate: load after iter-1 x
            nc.sync.dma_start(densel_s[:], densel.ap()[:, :])
            nc.sync.dma_start(bcsel_s[:], bcsel.ap()[:, :])
            nc.sync.dma_start(bgate_s[:], bgate_c.ap()[:, :])

        wg8_s = [wg_pool.tile([128, 2 * HID], FP8, tag=f"wg{kp}",
                              name=f"wg_{kp}") for kp in range(NK // 2)]
        wo_s = wo_pool.tile([128, NK, HID], BF16, tag="wo", name="wo")

        def load_rest():
            for kp in range(NK // 2):
                nc.gpsimd.dma_start(
                    wg8_s[kp][:, 0:HID],
                    wg8.ap()[256 * kp:256 * kp + 128, :])
                nc.gpsimd.dma_start(
                    wg8_s[kp][:, HID:2 * HID],
                    wg8.ap()[256 * kp + 128:256 * kp + 256, :])
            nc.gpsimd.dma_start(
                wo_s[:], woutT.ap()[:, :].rearrange("(k p) m -> p k m", p=128))

        # ---- helpers -----------------------------------------------------
        def emit_x8part(i, split=False):
            """fp8 DoubleRow-packed x tiles [128, 2*WG] for the q,k
            sections; plain 2D DMAs (contiguous rows).  split=True rides
            half the tiles on the scalar queue (startup only)."""
            tok = slice(HALO + (i - 1) * WG, HALO + i * WG)
            x8s = []
            for kp in range(NK // 2):
                t = x8_pool.tile([128, 2 * WG], FP8, tag="x8",
                                 name=f"x8_{i}_{kp}")
                eng = nc.scalar if split and kp >= 2 else nc.sync
                eng.dma_start(
                    t[:, 0:WG], xT8.ap()[256 * kp:256 * kp + 128, tok])
                eng.dma_start(
                    t[:, WG:2 * WG],
                    xT8.ap()[256 * kp + 128:256 * kp + 256, tok])
                x8s.append(t)
            return x8s

        def emit_xbpart(i, split=False):
            """bf16 x tiles feeding the v section."""
            tok = slice(HALO + (i - 1) * WG, HALO + i * WG)
            xts = []
            for k in range(NK):
                t = xt_pool.tile([128, WG], BF16, tag="xt", name=f"xt_{i}_{k}")
                eng = nc.scalar if split and k >= 4 else nc.sync
                eng.dma_start(t[:], xT.ap()[128 * k:128 * (k + 1), tok])
                xts.append(t)
            return xts

        def emit_x(i):
            x8s = emit_x8part(i)
            xts = emit_xbpart(i)
            return xts, x8s

        def emit_sec(i, sec, xts, drain):
            """bf16 v-section matmuls."""
            for j in range(NH):
                ps = ps_pool.tile([128, WG], F32, tag="mm",
                                  name=f"ps_{i}_{sec}_{j}")
                for k in range(NK):
                    nc.tensor.matmul(
                        ps[:], wv_s[k][:, 128 * j:128 * (j + 1)],
                        xts[k][:], start=(k == 0), stop=(k == NK - 1))
                drain(j, ps)

        def emit_sec8(i, sec, x8s, drain, w=WG):
            """fp8 DoubleRow q/k-section matmuls: 4 kp tiles of 256
            contraction each."""
            for j in range(NH):
                ps = ps_pool.tile([128, WG], F32, tag="mm",
                                  name=f"p8_{i}_{sec}_{j}")
                for kp in range(NK // 2):
                    lhs = wqk8_s[sec][kp][:, :].rearrange(
                        "p (i m) -> p i m", i=2)[:, :, 128 * j:128 * (j + 1)]
                    rhs = x8s[kp][:, 0:2 * w].rearrange(
                        "p (i n) -> p i n", i=2)
                    nc.tensor.matmul(ps[:, 0:w], lhs, rhs, start=(kp == 0),
                                     stop=(kp == NK // 2 - 1), perf_mode=DR)
                drain(j, ps)

        state = {"ks": [None] * NH, "kv": [None] * NH}

        def inv_ap(j):
            return decp_s[:, j:j + 1]

        def emit_chain(i, which, k1_i, v1_i, q1_i, pool, tag, koff, out_w,
                       tt_eng=None):
            """Full scan chain for one path.  Per tile: y = data *
            (1/d)^(t+1) (with fused k*v for the kv path), cs = carry +
            cumsum(y), out = q * cs, and the next-iter carry = cs[-1] *
            d^out_w.  The d^(t+1) rescale of the running sums cancels
            between the attention numerator (q*cs_kv) and denominator
            (q.cs_ks) -- both paths share the same pinv frame -- so out
            stays UNSCALED and the q* multiply is a plain tensor_tensor.
            Single-fold custom ops only: dual-fold specs run at half the
            DVE element rate."""
            outs = []
            g = 3 * NH if out_w == W1 else 2 * NH
            for j in range(NH):
                y = xps_pool.tile([128, W1], BF16, tag="xp",
                                  name=f"y{which}_{i}_{j}")
                carry = 0.0 if i == 1 else state[which][j][:, 0:1]
                if which == "kv":
                    nc.vector._custom_dve(
                        XP_OP, out=y[:, 0:out_w], in0=k1_i[j][:, 0:out_w],
                        in1=v1_i[j][:, 0:out_w], s0=inv_ap(j))
                else:
                    nc.vector._custom_dve(
                        XPK_OP, out=y[:, 0:out_w], in0=k1_i[j][:, 0:out_w],
                        s0=inv_ap(j))
                cum = cum_pool.tile([128, W1], BF16, tag=f"c{which}{j}",
                                    name=f"c{which}_{i}_{j}")
                nc.vector._custom_dve(CSUM_OP, out=cum[:, 0:out_w],
                                      in0=y[:, 0:out_w], s0=carry)
                o = pool.tile([128, WG], BF16, tag=tag, name=f"{tag}_{i}_{j}")
                (tt_eng or nc.vector).tensor_tensor(
                    o[:], q1_i[j][:], cum[:, koff:koff + WG], AL.mult)
                outs.append(o)
                if i < NG:
                    s = st_pool.tile([128, 1], F32, tag=f"s{which}{j}",
                                     name=f"s{which}_{i}_{j}")
                    nc.gpsimd.tensor_scalar_mul(
                        s[:], cum[:, out_w - 1:out_w],
                        decp_s[:, g + j:g + j + 1])
                    state[which][j] = s
            return outs

        def emit_den(i, prods):
            dps = psd_pool.tile([H, WG], F32, tag="den", name=f"dps_{i}")
            for j in range(NH):
                nc.tensor.matmul(dps[:], densel_s[:, H * j:H * (j + 1)],
                                 prods[j][:], start=(j == 0),
                                 stop=(j == NH - 1))
            nc.vector.tensor_scalar_max(dps[:], dps[:], 1e-6 * S * S)
            den_f = den_pool.tile([H, WG], F32, tag="denf", name=f"denf_{i}")
            nc.vector.reciprocal_approx_fast(den_f[:], dps[:])
            den_i = den_pool.tile([H, WG], BF16, tag="deni", bufs=2,
                                  name=f"deni_{i}")
            nc.vector.tensor_scalar_mul(den_i[:], den_f[:], 1.0)
            return den_i

        def emit_oa(p_qc, p_den, p_v1, p_i):
            """bc matmul (PE), oa=qc*bc (DVE), oa8 (ACT), dls (gpsimd) for
            iter p.  All inputs were produced in iter p."""
            oa8 = [oa8_pool.tile([128, 2 * WG], FP8, tag=f"o8{kp}",
                                 name=f"oa8_{p_i}_{kp}")
                   for kp in range(NH // 2)]
            dls = []
            for j in range(NH):
                bc = bc_pool.tile([128, WG], F32, tag="bc",
                                  name=f"bc_{p_i}_{j}")
                nc.tensor.matmul(bc[:], bcsel_s[:, 128 * j:128 * (j + 1)],
                                 p_den[:, :], start=True, stop=True)
                oa = oa_pool.tile([128, WG], BF16, tag="oa",
                                  name=f"oa_{p_i}_{j}")
                nc.vector.tensor_tensor(oa[:], p_qc[j][:], bc[:], AL.mult)
                nc.scalar.activation(
                    oa8[j // 2][:, WG * (j % 2):WG * (j % 2 + 1)],
                    oa[:], AF.Copy, scale=OSC)
                dl = dl_pool.tile([128, WG], BF16, tag="dl",
                                  name=f"dl_{p_i}_{j}")
                nc.gpsimd.tensor_tensor(dl[:], oa[:], p_v1[j], AL.subtract)
                dls.append(dl)
            return oa8, dls

        def emit_gate_mix(p_oa8, p_dls, p_v1, p_i, eng):
            mixes = []
            for ot in range(NH):
                ps = ps_pool.tile([128, WG], F32, tag="mm",
                                  name=f"gp_{p_i}_{ot}")
                for kp in range(NK // 2):
                    lhs = wg8_s[kp][:, :].rearrange(
                        "p (i m) -> p i m", i=2)[:, :, 128 * ot:128 * (ot + 1)]
                    rhs = p_oa8[kp][:, :].rearrange("p (i n) -> p i n", i=2)
                    nc.tensor.matmul(ps[:], lhs, rhs, start=(kp == 0),
                                     stop=(kp == NK // 2 - 1), perf_mode=DR)
                # g = sigmoid(GS*ps + b) as Exp(-GS*ps - b) then 1/(1+x):
                # bgate_s holds -b so the ACT stays on the Exp table
                eg = gt_pool.tile([128, WG], BF16, tag="eg",
                                  name=f"eg_{p_i}_{ot}")
                nc.scalar.activation(eg[:], ps[:], AF.Exp,
                                     bias=bgate_s[:, ot:ot + 1], scale=-GS)
                g = gt_pool.tile([128, WG], BF16, tag="gt",
                                 name=f"gt_{p_i}_{ot}")
                nc.vector._custom_dve(SIGR_OP, out=g[:], in0=eg[:],
                                      s0=-0.23549792, s1=2.0017324)
                eng.tensor_tensor(p_dls[ot][:], g[:], p_dls[ot][:], AL.mult)
                mx = mx_pool.tile([128, WG], BF16, tag="mx",
                                  name=f"mx_{p_i}_{ot}")
                eng.tensor_tensor(mx[:], p_dls[ot][:], p_v1[ot], AL.add)
                mixes.append(mx)
            return mixes

        def emit_out(mixes, p_i):
            out_tok = slice((p_i - 1) * WG, p_i * WG)
            for ot in range(NH):
                ps = ps_pool.tile([128, WG], F32, tag="mm",
                                  name=f"yp_{p_i}_{ot}")
                for k in range(NK):
                    nc.tensor.matmul(
                        ps[:], wo_s[:, k, 128 * ot:128 * (ot + 1)],
                        mixes[k][:], start=(k == 0), stop=(k == NK - 1))
                ysb = y_pool.tile([128, WG], BF16, tag="ysb",
                                  name=f"ysb_{p_i}_{ot}")
                nc.scalar.activation(ysb[:], ps[:], AF.Copy, scale=1.0 / S)
                nc.sync.dma_start(yT.ap()[128 * ot:128 * (ot + 1), out_tok],
                                  ysb[:])

        # ================= prologue: halo k-section ======================
        # DMA priority: the PE's first deps (k-weights on the tensor queue,
        # halo x8 + iter-1 x8 on sync) land first; bf16 x (v section, needed
        # a section later) next; late consts after.  v-weights ride the idle
        # scalar queue, q-weights/gate/out the gpsimd queue.
        load_wqk8_sec(1, nc.scalar)  # k-section weights, on the idle ACT queue
        xh8 = []
        for kp in range(NK // 2):
            t = x8_pool.tile([128, 2 * HALO], FP8, tag="xh8", bufs=4,
                             name=f"xh8_{kp}")
            eng = nc.sync if kp < 2 else nc.scalar
            eng.dma_start(t[:, 0:HALO],
                          xT8.ap()[256 * kp:256 * kp + 128, 0:HALO])
            eng.dma_start(t[:, HALO:2 * HALO],
                          xT8.ap()[256 * kp + 128:256 * kp + 256, 0:HALO])
            xh8.append(t)
        x1_8 = emit_x8part(1)
        x1_b = emit_xbpart(1)
        xh = []
        for k in range(NK):
            t = xt_pool.tile([128, HALO], BF16, tag="xh", bufs=8,
                             name=f"xh_{k}")
            nc.sync.dma_start(t[:], xT.ap()[128 * k:128 * (k + 1), 0:HALO])
            xh.append(t)
        x1 = (x1_b, x1_8)
        load_dec_consts()
        load_late_consts()
        load_wqk8_sec(0, nc.gpsimd)  # q-section
        load_wv(nc.gpsimd)           # v-section weights

        k1_1 = [k1_pool.tile([128, W1], BF16, tag="k1", name=f"k1_1_{j}")
                for j in range(NH)]
        v1_1 = [v1_pool.tile([128, W1], BF16, tag="v1", name=f"v1_1_{j}")
                for j in range(NH)]

        def emit_halo_k():
            """kp-outer so the PE starts on the first weight/x tile the
            moment it lands (all 8 psum banks are free at startup)."""
            pss = []
            for j in range(NH):
                pool, tag = (ps_pool, "mm") if j < 5 else (bc_pool, "bc")
                if j == 7:
                    pool, tag = psd_pool, "den"
                pss.append(pool.tile([128, WG], F32, tag=tag,
                                     name=f"psh_k_{j}"))
            for kp in range(NK // 2):
                lhs = wqk8_s[1][kp][:, :].rearrange(
                    "p (i m) -> p i m", i=2)
                rhs = xh8[kp][:, :].rearrange("p (i n) -> p i n", i=2)
                for j in range(NH):
                    nc.tensor.matmul(
                        pss[j][:, 0:HALO], lhs[:, :, 128 * j:128 * (j + 1)],
                        rhs, start=(kp == 0), stop=(kp == NK // 2 - 1),
                        perf_mode=DR)
            for j in range(NH):
                ps = pss[j]
                e = et_pool.tile([128, HALO], BF16, tag="kr", bufs=1,
                                 name=f"eh_{j}")
                nc.scalar.activation(e[:], ps[:, 0:HALO], AF.Exp,
                                     scale=1.0 / S)
                kr = et_pool.tile([128, HALO], BF16, tag="kr2", bufs=1,
                                  name=f"krh_{j}")
                nc.vector._custom_dve(PHI_OP, out=kr[:], in0=ps[:, 0:HALO],
                                      in1=e[:], s0=S)
                # mask: half-0 cores zero the halo k (scan state 0 at t=0)
                nc.vector.tensor_scalar_mul(k1_1[j][:, 0:HALO], kr[:],
                                            mask_s[:, 0:1])
        emit_halo_k()

        def emit_halo_v():
            """Halo v-section, drained into v1_1[:, 0:HALO].  Emitted next
            to iter 1's v-section so it never stalls startup on the
            v-weight DMA."""
            for j in range(NH):
                ps = ps_pool.tile([128, WG], F32, tag="mm",
                                  name=f"psh_v_{j}")
                for k in range(NK):
                    nc.tensor.matmul(ps[:, 0:HALO],
                                     wv_s[k][:, 128 * j:128 * (j + 1)],
                                     xh[k][:], start=(k == 0),
                                     stop=(k == NK - 1))
                nc.scalar.copy(v1_1[j][:, 0:HALO], ps[:, 0:HALO])
        load_rest()

        # ================= main loop =====================================
        # prev = (qc, den_i, v1w, i): produced in iter i, consumed in i+1
        prev = None
        xs = {1: x1, 2: emit_x(2)}
        for i in range(1, NG + 1):
            koff = HALO if i == 1 else 0
            w1 = W1 if i == 1 else WG
            xts, x8s = xs.pop(i)
            if 2 <= i < NG:
                xs[i + 1] = emit_x(i + 1)

            if i == 1:
                k1_i, v1_i = k1_1, v1_1
            else:
                k1_i = [k1_pool.tile([128, W1], BF16, tag="k1",
                                     name=f"k1_{i}_{j}") for j in range(NH)]
                v1_i = [v1_pool.tile([128, WG], BF16, tag="v1",
                                     name=f"v1_{i}_{j}") for j in range(NH)]
            # out-token views of v (skipping the halo columns in iter 1)
            v1w = [v1_i[j][:, koff:koff + WG] for j in range(NH)]

            # PE: bc matmuls for prev iter (tiny, inputs a full iter old);
            # DVE: oa; ACT: oa8 cast; gpsimd: dls
            if prev is not None:
                p_qc, p_den, p_v1w, p_i = prev
                oa8, dls = emit_oa(p_qc, p_den, p_v1w, p_i)

            # PE: k-section; DVE/ACT: phi-k drains
            def drain_k(j, ps, k1_i=k1_i, koff=koff, i=i):
                kview = k1_i[j][:, koff:koff + WG]
                e = et_pool.tile([128, WG], BF16, tag="e", name=f"e_k{i}_{j}")
                nc.scalar.activation(e[:], ps[:], AF.Exp, scale=1.0 / S)
                nc.vector._custom_dve(PHI_OP, out=kview, in0=ps[:], in1=e[:],
                                      s0=S)
            emit_sec8(i, 1, x8s, drain_k)

            # PE: q-section; phi-q drains
            q1_i = [q1_pool.tile([128, WG], BF16, tag="q1",
                                 name=f"q1_{i}_{j}") for j in range(NH)]

            def drain_q(j, ps, q1_i=q1_i, i=i):
                e = et_pool.tile([128, WG], BF16, tag="e", name=f"e_q{i}_{j}")
                nc.scalar.activation(e[:], ps[:], AF.Exp, scale=1.0 / S)
                nc.vector._custom_dve(PHI_OP, out=q1_i[j][:], in0=ps[:],
                                      in1=e[:], s0=S)
            emit_sec8(i, 0, x8s, drain_q)

            # PE: gate matmul for prev iter (fp8 DoubleRow); sigmoid; mix
            if prev is not None:
                mixes = emit_gate_mix(oa8, dls, p_v1w, p_i, nc.vector)

            # DVE: ksum scan chain; prods (= q * cs_ks) on gpsimd -- its
            # consumer (den matmul -> bc in iter i+1) has a full iteration
            # of slack, so the slower engine's latency is hidden
            prods = emit_chain(i, "ks", k1_i, None, q1_i, pr_pool, "pr",
                               koff, w1, tt_eng=nc.gpsimd)

            # PE: v-section; ACT copies
            if i == 1:
                emit_halo_v()

            def drain_v(j, ps, v1w=v1w):
                nc.scalar.copy(v1w[j], ps[:])
            emit_sec(i, 2, xts, drain_v)

            # PE: den selector matmul; DVE: clip, recip, cast
            den_i = emit_den(i, prods)

            # PE: out matmul for prev iter; ACT y copies; DMA out
            if prev is not None:
                emit_out(mixes, p_i)

            # DVE: kv scan chain (fused k*v) + qc.  qc rides gpsimd except
            # in the last iteration, whose qc feeds the epilogue tail with
            # no slack.
            qcs = emit_chain(i, "kv", k1_i, v1_i, q1_i, qc_pool, "qc",
                             koff, w1,
                             tt_eng=None if i == NG else nc.gpsimd)

            if i == NG:
                oa8_l, dls_l = emit_oa(qcs, den_i, v1w, i)
            prev = (qcs, den_i, v1w, i)

        # ================= epilogue ======================================
        _, _, p_v1w, p_i = prev
        mixes = emit_gate_mix(oa8_l, dls_l, p_v1w, p_i, nc.vector)
        emit_out(mixes, p_i)

    nc.compile()
    return nc


def _sigmoid(v):
    return 1.0 / (1.0 + np.exp(-v))


def _make_inputs(x, Wqkv, Wout, Wgate, bgate, decay_param):
    decay = _sigmoid(np.asarray(decay_param, np.float64)).astype(np.float32)
    bf = ml_dtypes.bfloat16
    f8 = ml_dtypes.float8_e4m3
    # x32: the whole pipeline rides this scale (see module docstring);
    # scaling by a power of two is exact in bf16.
    wqkvT_f = np.asarray(Wqkv, np.float32).T * np.float32(S)  # [HID, 3*HID]
    # q,k sections in fp8, DoubleRow-packed: [512*sec+128*kp+p, i*HID+m]
    # = wqkvT_f[256*kp+128*i+p, HID*sec+m]
    wqk8 = np.empty((HID, 2 * HID), np.float32)
    for sec in range(2):
        blk = wqkvT_f[:, HID * sec:HID * (sec + 1)]  # [1024, 1024]
        wqk8[512 * sec:512 * (sec + 1)] = (
            blk.reshape(NK // 2, 2, 128, HID).transpose(0, 2, 1, 3)
            .reshape(512, 2 * HID))
    wqk8 = np.ascontiguousarray(wqk8).astype(f8)
    wvT = np.ascontiguousarray(wqkvT_f[:, 2 * HID:3 * HID]).astype(bf)
    wg8 = np.ascontiguousarray(
        np.asarray(Wgate, np.float32).T * np.float32(S)).astype(f8)
    woutT = np.ascontiguousarray(np.asarray(Wout, np.float32).T).astype(bf)

    p = np.arange(128)
    dec_c = np.empty((128, NH), np.float32)
    for j in range(NH):
        dec_c[:, j] = decay[2 * j + p // 64]
    # decay powers for the cumsum-form scans: 1/d, d^HALO, d^WG, d^W1
    dec64 = np.empty((128, NH), np.float64)
    for j in range(NH):
        dec64[:, j] = decay.astype(np.float64)[2 * j + p // 64]
    decp_c = np.concatenate([1.0 / dec64, dec64 ** HALO, dec64 ** WG,
                             dec64 ** W1], axis=1).astype(np.float32)
    densel = np.zeros((128, NH * H), np.float32)
    for j in range(NH):
        for pp in range(128):
            densel[pp, H * j + 2 * j + pp // 64] = 1.0
    bcsel = np.zeros((H, NH * 128), np.float32)
    for j in range(NH):
        for m in range(128):
            bcsel[2 * j + m // 64, 128 * j + m] = 1.0
    # negated: the kernel computes the gate as Exp(-GS*ps - b) + 1/(1+x)
    bgate_c = np.ascontiguousarray(
        -np.asarray(bgate, np.float32).reshape(NH, 128).T)

    in_maps = []
    for c in range(8):
        b, half = c // 2, c % 2
        xb = np.asarray(x[b], np.float32)  # [T, HID]
        if half == 0:
            xloc = np.concatenate(
                [np.zeros((HALO, HID), np.float32), xb[:HALF_T]], axis=0)
            mask = np.zeros((128, 1), np.float32)
        else:
            xloc = xb[HALF_T - HALO:]
            mask = np.ones((128, 1), np.float32)
        xlocT = np.ascontiguousarray(xloc.T)
        in_maps.append({
            "xT": xlocT.astype(bf), "xT8": xlocT.astype(f8),
            "wqk8": wqk8, "wvT": wvT, "wg8": wg8, "woutT": woutT,
            "dec_c": dec_c, "decp_c": decp_c, "mask_c": mask,
            "densel": densel.astype(bf), "bcsel": bcsel.astype(bf),
            "bgate_c": bgate_c,
        })
    return in_maps


def kernel(x, Wqkv, Wout, Wgate, bgate, decay_param):
    if "nc" not in _cache:
        _cache["nc"] = _build_nc()
    nc = _cache["nc"]
    in_maps = _make_inputs(x, Wqkv, Wout, Wgate, bgate, decay_param)
    res = run_bass_kernel_spmd(nc, in_maps, list(range(8)))
    y = np.empty((B, T, HID), np.float32)
    for c in range(8):
        b, half = c // 2, c % 2
        y[b, half * HALF_T:(half + 1) * HALF_T, :] = res.results[c]["yT"].T
    return y

